# revision 36
# baseline (speedup 1.0000x reference)
"""Conv2d(128->256, 3x3, pad 1) with LoRA (rank 8) — Trainium2 Bass kernel.

Strategy (v4):
  - Data-parallel over batch: 16 images -> 2 per core x 8 cores; weights
    replicated.
  - The LoRA delta folds into the conv weight on the host (weight
    preprocessing, 2.4 MFLOP of the 38.7 GFLOP total):
        W_eff = W + (alpha/rank) * (B @ A).reshape(C_OUT, C_IN, 3, 3)
    so the device runs one homogeneous conv stream.
  - The 3x3 conv = 9 shifted matmuls accumulating in PSUM:
        out[co, pix] += W_eff[co, :, kh, kw]^T @ x_shift[ci, pix]
    K = C_IN = 128 (partition dim), M = 128 (co block), N = 512 (8 rows x
    64 cols), bf16. PE floor: 288 x ~216ns = 62.3us.
  - All inputs arrive bf16 from the host: no on-device casts; fp32 on the
    PE only in the two HAM warmup matmuls.
  - Startup levers (measured): main starts ~6.1us (fixed preamble); each
    dma_start costs ~0.6-0.8us sequencer dispatch; first DMA data ~8.6us;
    the rings round-robin every queued transfer, so the bulk (late-needed)
    DMAs are chained behind early conv matmuls to keep the critical
    weff/x-chunk completions fast.
  - The PE queue is strict FIFO and the scheduler may hoist a not-ready
    matmul ahead of ready ones, stalling everything: every PE matmul is
    chained in emission order.
  - HAM: the PE runs at 1.2 GHz until ~3.4-4.5us of sustained "real"
    activity. Two fp32 N=512 warmup matmuls (4 HI/LO slices, 3.4us) start
    the accumulation at ~7.2us with no DMA deps; the conv continues it.
    (bf16 N=256 warmups and K=8-partition matmuls measurably do NOT
    advance the HAM accumulation — keep them out of the stream.)
  - PSUM drains (fused bias add, bf16 out) alternate ACT / DVE; output is
    bf16 (halves store traffic); host converts back to f32.
"""

import numpy as np
import ml_dtypes

import concourse.bass as bass
import concourse.tile as tile
from concourse.tile import add_dep_helper
from concourse import bacc, mybir
from concourse.bass_utils import run_bass_kernel_spmd

N_CORES = 8
B, C_IN, H, W_DIM = 16, 128, 64, 64
C_OUT = 256
RANK = 8
SCALING = 2.0  # alpha/rank = 16/8
HP, WP = H + 2, W_DIM + 2  # zero-padded image dims (66x66)
B_LOC = B // N_CORES  # images per core
NPIX = H * W_DIM  # 4096
ROWS_PER_TILE = 8  # output rows per matmul group -> N = 8*64 = 512
N_RG = H // ROWS_PER_TILE  # 8 row groups
N_CHUNK = 6  # x DMA chunks per image
# chunk 0 = exactly the 10 padded rows rg0 needs (smallest critical DMA);
# the rest uniform. rg r needs rows 8r..8r+9 — each is covered by two
# adjacent chunks.
XB = [0, 660, 1452, 2178, 2904, 3630, 4356]
N_WARM = 2  # fp32 N=512 warmup matmuls (2 HI/LO slices each)

F32 = mybir.dt.float32
BF16 = mybir.dt.bfloat16
IDENT = mybir.ActivationFunctionType.Identity


def _build_nc():
    nc = bacc.Bacc(
        "TRN2",
        target_bir_lowering=False,
        debug=False,
        num_devices=N_CORES,
    )

    xp = nc.dram_tensor("xp", [B_LOC, C_IN, HP * WP], BF16, kind="ExternalInput").ap()
    we = nc.dram_tensor("we", [C_IN, 9 * C_OUT], BF16, kind="ExternalInput").ap()
    bv = nc.dram_tensor("bv", [128, 2], F32, kind="ExternalInput").ap()
    out = nc.dram_tensor("out", [B_LOC, C_OUT, NPIX], BF16, kind="ExternalOutput").ap()

    with tile.TileContext(nc) as tc:
        with (
            tc.tile_pool(name="persist", bufs=1) as persist,
            tc.tile_pool(name="outp", bufs=4) as outp,
            tc.tile_pool(name="psum", bufs=1, space="PSUM") as psum,
        ):
            # --- persistent SBUF tiles -------------------------------------
            x_sb = [
                persist.tile([C_IN, HP * WP], BF16, name=f"x_sb{i}")
                for i in range(B_LOC)
            ]
            weff = persist.tile([C_IN, 9 * C_OUT], BF16, name="weff")
            b_sb = persist.tile([128, 2], F32, name="b_sb")
            warm_sb = persist.tile([128, 512], F32, name="warm_sb")

            # --- explicit PE ordering --------------------------------------
            pe_link = [None]

            def pe_mm(*args, **kwargs):
                inst = nc.tensor.matmul(*args, **kwargs)
                if pe_link[0] is not None:
                    add_dep_helper(
                        inst.ins, pe_link[0].ins, sync=False, reason="PE total order"
                    )
                pe_link[0] = inst
                return inst

            # --- PE warm-up ------------------------------------------------
            nc.gpsimd.memset(warm_sb[:], 0.0)
            warm_ps = psum.tile([128, 512], F32, tag="warm", bufs=1, name="warm_ps")
            for _ in range(N_WARM):
                pe_mm(warm_ps[:], warm_sb[:, :128], warm_sb[:], start=True, stop=True)

            # --- input DMAs (critical first; bulk chained later) -----------
            def xdma(eng, i, c, after=None):
                lo, hi = XB[c], XB[c + 1]
                inst = eng.dma_start(x_sb[i][:, lo:hi], xp[i, :, lo:hi])
                if after is not None:
                    add_dep_helper(
                        inst.ins, after.ins, sync=True, reason="defer bulk DMA"
                    )
                return inst

            def wdma(eng, lo, hi):
                eng.dma_start(weff[:, lo:hi], we[:, lo:hi])

            nc.gpsimd.dma_start(b_sb[:], bv)
            xdma(nc.scalar, 0, 0)
            wdma(nc.sync, 0, 512)  # k0,k1
            wdma(nc.scalar, 512, 768)  # k2 alone: tiny, lands before its deadline
            wdma(nc.scalar, 768, 1280)  # k3,k4
            wdma(nc.sync, 1280, 9 * C_OUT)  # k5..k8
            xdma(nc.scalar, 0, 1)

            # --- the conv: 9 accumulating shift-matmuls per output tile ----
            N_TILES = B_LOC * 2 * N_RG  # 32

            def conv_mm(ps, img, cb, rg, k, half=None):
                dh, dw = k // 3 - 1, k % 3 - 1
                x_r = x_sb[img][:].rearrange("p (h w) -> p h w", w=WP)
                h0 = rg * ROWS_PER_TILE
                rows = ROWS_PER_TILE
                po = ps[:]
                if half is not None:
                    h0 += half * (ROWS_PER_TILE // 2)
                    rows = ROWS_PER_TILE // 2
                rhs = x_r[
                    :,
                    h0 + 1 + dh : h0 + 1 + dh + rows,
                    1 + dw : 65 + dw,
                ]
                lhsT = weff[:, k * 256 + cb * 128 : k * 256 + cb * 128 + 128]
                return pe_mm(po, lhsT, rhs, start=(k == 0), stop=(k == 8))

            def drain(ps, img, cb, rg, ti, half=None):
                o = outp.tile([128, 512], BF16, tag="o", name=f"o{ti}_{half}")
                dst = out[img, cb * 128 : (cb + 1) * 128, rg * 512 : (rg + 1) * 512]
                if half is not None:
                    sl = slice(half * 256, (half + 1) * 256)
                    if half == 0:
                        nc.scalar.activation(
                            o[:, sl], ps[:, :256], IDENT, bias=b_sb[:, cb : cb + 1]
                        )
                        nc.scalar.dma_start(dst[:, sl], o[:, sl])
                    else:
                        # very last tile: quarter drains on DVE+ACT in
                        # parallel, stores split across both HW queues
                        nc.vector.tensor_scalar_add(
                            o[:, 256:384], ps[:, :128], b_sb[:, cb : cb + 1]
                        )
                        nc.scalar.activation(
                            o[:, 384:512], ps[:, 128:256], IDENT,
                            bias=b_sb[:, cb : cb + 1],
                        )
                        nc.sync.dma_start(dst[:, 256:384], o[:, 256:384])
                        nc.scalar.dma_start(dst[:, 384:512], o[:, 384:512])
                elif ti == N_TILES - 2:
                    # drain halves on ACT+DVE in parallel, DMA on both queues
                    nc.scalar.activation(
                        o[:, :256], ps[:, :256], IDENT, bias=b_sb[:, cb : cb + 1]
                    )
                    nc.vector.tensor_scalar_add(
                        o[:, 256:], ps[:, 256:], b_sb[:, cb : cb + 1]
                    )
                    nc.sync.dma_start(dst[:, :256], o[:, :256])
                    nc.scalar.dma_start(dst[:, 256:], o[:, 256:])
                else:
                    if ti % 2 == 0:
                        nc.scalar.activation(
                            o[:], ps[:], IDENT, bias=b_sb[:, cb : cb + 1]
                        )
                    else:
                        nc.vector.tensor_scalar_add(o[:], ps[:], b_sb[:, cb : cb + 1])
                    q = nc.sync if ti % 2 == 0 else nc.scalar
                    q.dma_start(dst, o[:])

            # cb0/cb1 of each row-group interleaved: both use the same weff
            # chunk k and the same x rows, so the per-k weff-DMA deadline
            # pace halves at zero extra PE/DMA cost (a late weff chunk
            # would stall the PE >1.3us and trigger a HAM re-throttle).
            ti = 0
            for img in range(B_LOC):
                for rg in range(N_RG):
                    last_pair = (img, rg) == (B_LOC - 1, N_RG - 1)
                    ps0 = psum.tile([128, 512], F32, tag="cps", bufs=5, name=f"cps{ti}")
                    if last_pair:
                        # final cb1 tile as two N=256 half-groups in their
                        # own PSUM tiles: half 0's drain+DMA overlap half
                        # 1's matmuls (N=256 streams ~half the cycles)
                        for k in range(9):
                            conv_mm(ps0, img, 0, rg, k)
                        drain(ps0, img, 0, rg, ti)
                        for half in range(2):
                            ph = psum.tile(
                                [128, 256], F32, tag=f"cpsh{half}",
                                bufs=1, name=f"cpsh{half}",
                            )
                            for k in range(9):
                                conv_mm(ph, img, 1, rg, k, half=half)
                            drain(ph, img, 1, rg, ti + 1, half=half)
                        ti += 2
                        continue
                    ps1 = psum.tile(
                        [128, 512], F32, tag="cps", bufs=5, name=f"cps{ti + 1}"
                    )
                    for k in range(9):
                        mm = conv_mm(ps0, img, 0, rg, k)
                        conv_mm(ps1, img, 1, rg, k)
                        # stage the bulk DMAs off early conv matmuls so they
                        # don't steal ring bandwidth at startup
                        if ti == 0 and k == 4:
                            xdma(nc.sync, 0, 2, after=mm)
                            xdma(nc.scalar, 0, 3, after=mm)
                        elif ti == 2 and k == 0:
                            xdma(nc.sync, 0, 4, after=mm)
                            xdma(nc.scalar, 0, 5, after=mm)
                        elif ti == 4 and k == 0:
                            for c in range(N_CHUNK):
                                xdma(nc.gpsimd, 1, c, after=mm)
                    drain(ps0, img, 0, rg, ti)
                    drain(ps1, img, 1, rg, ti + 1)
                    ti += 2

    nc.compile()
    return nc


_NC_CACHE = None


def _get_nc():
    global _NC_CACHE
    if _NC_CACHE is None:
        _NC_CACHE = _build_nc()
    return _NC_CACHE


def _host_prep(x, W, b, lora_A, lora_B):
    """Host input staging: pad + transpose + bf16 cast + LoRA weight fold."""
    bf16 = ml_dtypes.bfloat16
    x = np.asarray(x, dtype=np.float32)
    xp_all = np.zeros((B, C_IN, HP, WP), dtype=bf16)
    xp_all[:, :, 1 : H + 1, 1 : W_DIM + 1] = x.astype(bf16)
    xp_all = xp_all.reshape(B, C_IN, HP * WP)

    # W_eff = W + 2*(B@A), then [co, ci*9+k] -> [ci, k, co]
    weff = np.asarray(W, dtype=np.float32).reshape(C_OUT, C_IN * 9) + SCALING * (
        np.asarray(lora_B, dtype=np.float32) @ np.asarray(lora_A, dtype=np.float32)
    )
    we = np.ascontiguousarray(
        weff.reshape(C_OUT, C_IN, 9).transpose(1, 2, 0)
    ).reshape(C_IN, 9 * C_OUT).astype(bf16)
    # [256] -> [128, 2]: bv[p, cb] = b[cb*128 + p]
    bv = np.ascontiguousarray(np.asarray(b, dtype=np.float32).reshape(2, 128).T)
    return xp_all, we, bv


def run(x, W, b, lora_A, lora_B, trace=False):
    """Run the kernel on 8 cores; returns (full_output, BassKernelResults)."""
    xp_all, we, bv = _host_prep(x, W, b, lora_A, lora_B)
    nc = _get_nc()
    in_maps = []
    for c in range(N_CORES):
        in_maps.append(
            {
                "xp": np.ascontiguousarray(xp_all[c * B_LOC : (c + 1) * B_LOC]),
                "we": we,
                "bv": bv,
            }
        )
    res = run_bass_kernel_spmd(
        nc, in_maps, core_ids=list(range(N_CORES)), trace=trace
    )
    out = np.concatenate(
        [np.asarray(r["out"]).astype(np.float32) for r in res.results], axis=0
    )
    return out.reshape(B, C_OUT, H, W_DIM), res


def kernel(x, W, b, lora_A, lora_B):
    out, _ = run(x, W, b, lora_A, lora_B, trace=False)
    return out


# revision 37
# speedup vs baseline: 1.0013x; 1.0013x over previous
"""Conv2d(128->256, 3x3, pad 1) with LoRA (rank 8) — Trainium2 Bass kernel.

Strategy (v4):
  - Data-parallel over batch: 16 images -> 2 per core x 8 cores; weights
    replicated.
  - The LoRA delta folds into the conv weight on the host (weight
    preprocessing, 2.4 MFLOP of the 38.7 GFLOP total):
        W_eff = W + (alpha/rank) * (B @ A).reshape(C_OUT, C_IN, 3, 3)
    so the device runs one homogeneous conv stream.
  - The 3x3 conv = 9 shifted matmuls accumulating in PSUM:
        out[co, pix] += W_eff[co, :, kh, kw]^T @ x_shift[ci, pix]
    K = C_IN = 128 (partition dim), M = 128 (co block), N = 512 (8 rows x
    64 cols), bf16. PE floor: 288 x ~216ns = 62.3us.
  - All inputs arrive bf16 from the host: no on-device casts; fp32 on the
    PE only in the two HAM warmup matmuls.
  - Startup levers (measured): main starts ~6.1us (fixed preamble); each
    dma_start costs ~0.6-0.8us sequencer dispatch; first DMA data ~8.6us;
    the rings round-robin every queued transfer, so the bulk (late-needed)
    DMAs are chained behind early conv matmuls to keep the critical
    weff/x-chunk completions fast.
  - The PE queue is strict FIFO and the scheduler may hoist a not-ready
    matmul ahead of ready ones, stalling everything: every PE matmul is
    chained in emission order.
  - HAM: the PE runs at 1.2 GHz until ~3.4-4.5us of sustained "real"
    activity. Two fp32 N=512 warmup matmuls (4 HI/LO slices, 3.4us) start
    the accumulation at ~7.2us with no DMA deps; the conv continues it.
    (bf16 N=256 warmups and K=8-partition matmuls measurably do NOT
    advance the HAM accumulation — keep them out of the stream.)
  - PSUM drains (fused bias add, bf16 out) alternate ACT / DVE; output is
    bf16 (halves store traffic); host converts back to f32.
"""

import numpy as np
import ml_dtypes

import concourse.bass as bass
import concourse.tile as tile
from concourse.tile import add_dep_helper
from concourse import bacc, mybir
from concourse.bass_utils import run_bass_kernel_spmd

N_CORES = 8
B, C_IN, H, W_DIM = 16, 128, 64, 64
C_OUT = 256
RANK = 8
SCALING = 2.0  # alpha/rank = 16/8
HP, WP = H + 2, W_DIM + 2  # zero-padded image dims (66x66)
B_LOC = B // N_CORES  # images per core
NPIX = H * W_DIM  # 4096
ROWS_PER_TILE = 8  # output rows per matmul group -> N = 8*64 = 512
N_RG = H // ROWS_PER_TILE  # 8 row groups
N_CHUNK = 6  # x DMA chunks per image
# chunk 0 = exactly the 10 padded rows rg0 needs (smallest critical DMA);
# the rest uniform. rg r needs rows 8r..8r+9 — each is covered by two
# adjacent chunks.
XB = [0, 660, 1452, 2178, 2904, 3630, 4356]
N_WARM = 2  # fp32 N=512 warmup matmuls (2 HI/LO slices each)

F32 = mybir.dt.float32
BF16 = mybir.dt.bfloat16
IDENT = mybir.ActivationFunctionType.Identity


def _build_nc():
    nc = bacc.Bacc(
        "TRN2",
        target_bir_lowering=False,
        debug=False,
        num_devices=N_CORES,
    )

    xp = nc.dram_tensor("xp", [B_LOC, C_IN, HP * WP], BF16, kind="ExternalInput").ap()
    we = nc.dram_tensor("we", [C_IN, 9 * C_OUT], BF16, kind="ExternalInput").ap()
    bv = nc.dram_tensor("bv", [128, 2], F32, kind="ExternalInput").ap()
    out = nc.dram_tensor("out", [B_LOC, C_OUT, NPIX], BF16, kind="ExternalOutput").ap()

    with tile.TileContext(nc) as tc:
        with (
            tc.tile_pool(name="persist", bufs=1) as persist,
            tc.tile_pool(name="outp", bufs=4) as outp,
            tc.tile_pool(name="psum", bufs=1, space="PSUM") as psum,
        ):
            # --- persistent SBUF tiles -------------------------------------
            x_sb = [
                persist.tile([C_IN, HP * WP], BF16, name=f"x_sb{i}")
                for i in range(B_LOC)
            ]
            weff = persist.tile([C_IN, 9 * C_OUT], BF16, name="weff")
            b_sb = persist.tile([128, 2], F32, name="b_sb")
            warm_sb = persist.tile([128, 512], F32, name="warm_sb")

            # --- explicit PE ordering --------------------------------------
            pe_link = [None]

            def pe_mm(*args, **kwargs):
                inst = nc.tensor.matmul(*args, **kwargs)
                if pe_link[0] is not None:
                    add_dep_helper(
                        inst.ins, pe_link[0].ins, sync=False, reason="PE total order"
                    )
                pe_link[0] = inst
                return inst

            # --- PE warm-up ------------------------------------------------
            # memset on DVE: the gpsimd memset gets chunked and interleaved
            # with queue bookkeeping (finishes ~7.7us); DVE is otherwise
            # idle here and completes it ~1us earlier, so the warmup (and
            # with it the whole conv stream) starts ~1us sooner.
            nc.vector.memset(warm_sb[:], 0.0)
            warm_ps = psum.tile([128, 512], F32, tag="warm", bufs=1, name="warm_ps")
            for _ in range(N_WARM):
                pe_mm(warm_ps[:], warm_sb[:, :128], warm_sb[:], start=True, stop=True)

            # --- input DMAs (critical first; bulk chained later) -----------
            def xdma(eng, i, c, after=None):
                lo, hi = XB[c], XB[c + 1]
                inst = eng.dma_start(x_sb[i][:, lo:hi], xp[i, :, lo:hi])
                if after is not None:
                    add_dep_helper(
                        inst.ins, after.ins, sync=True, reason="defer bulk DMA"
                    )
                return inst

            def wdma(eng, lo, hi):
                eng.dma_start(weff[:, lo:hi], we[:, lo:hi])

            nc.gpsimd.dma_start(b_sb[:], bv)
            xdma(nc.scalar, 0, 0)
            wdma(nc.sync, 0, 512)  # k0,k1
            wdma(nc.scalar, 512, 768)  # k2 alone: tiny, lands before its deadline
            wdma(nc.scalar, 768, 1280)  # k3,k4
            wdma(nc.sync, 1280, 9 * C_OUT)  # k5..k8
            xdma(nc.scalar, 0, 1)

            # --- the conv: 9 accumulating shift-matmuls per output tile ----
            N_TILES = B_LOC * 2 * N_RG  # 32

            def conv_mm(ps, img, cb, rg, k, half=None):
                dh, dw = k // 3 - 1, k % 3 - 1
                x_r = x_sb[img][:].rearrange("p (h w) -> p h w", w=WP)
                h0 = rg * ROWS_PER_TILE
                rows = ROWS_PER_TILE
                po = ps[:]
                if half is not None:
                    h0 += half * (ROWS_PER_TILE // 2)
                    rows = ROWS_PER_TILE // 2
                rhs = x_r[
                    :,
                    h0 + 1 + dh : h0 + 1 + dh + rows,
                    1 + dw : 65 + dw,
                ]
                lhsT = weff[:, k * 256 + cb * 128 : k * 256 + cb * 128 + 128]
                return pe_mm(po, lhsT, rhs, start=(k == 0), stop=(k == 8))

            def drain(ps, img, cb, rg, ti, half=None):
                o = outp.tile([128, 512], BF16, tag="o", name=f"o{ti}_{half}")
                dst = out[img, cb * 128 : (cb + 1) * 128, rg * 512 : (rg + 1) * 512]
                if half is not None:
                    sl = slice(half * 256, (half + 1) * 256)
                    if half == 0:
                        nc.scalar.activation(
                            o[:, sl], ps[:, :256], IDENT, bias=b_sb[:, cb : cb + 1]
                        )
                        nc.scalar.dma_start(dst[:, sl], o[:, sl])
                    else:
                        # very last tile: quarter drains on DVE+ACT in
                        # parallel, stores split across both HW queues
                        nc.vector.tensor_scalar_add(
                            o[:, 256:384], ps[:, :128], b_sb[:, cb : cb + 1]
                        )
                        nc.scalar.activation(
                            o[:, 384:512], ps[:, 128:256], IDENT,
                            bias=b_sb[:, cb : cb + 1],
                        )
                        nc.sync.dma_start(dst[:, 256:384], o[:, 256:384])
                        nc.scalar.dma_start(dst[:, 384:512], o[:, 384:512])
                elif ti == N_TILES - 2:
                    # drain halves on ACT+DVE in parallel, DMA on both queues
                    nc.scalar.activation(
                        o[:, :256], ps[:, :256], IDENT, bias=b_sb[:, cb : cb + 1]
                    )
                    nc.vector.tensor_scalar_add(
                        o[:, 256:], ps[:, 256:], b_sb[:, cb : cb + 1]
                    )
                    nc.sync.dma_start(dst[:, :256], o[:, :256])
                    nc.scalar.dma_start(dst[:, 256:], o[:, 256:])
                else:
                    if ti % 2 == 0:
                        nc.scalar.activation(
                            o[:], ps[:], IDENT, bias=b_sb[:, cb : cb + 1]
                        )
                    else:
                        nc.vector.tensor_scalar_add(o[:], ps[:], b_sb[:, cb : cb + 1])
                    q = nc.sync if ti % 2 == 0 else nc.scalar
                    q.dma_start(dst, o[:])

            # cb0/cb1 of each row-group interleaved: both use the same weff
            # chunk k and the same x rows, so the per-k weff-DMA deadline
            # pace halves at zero extra PE/DMA cost (a late weff chunk
            # would stall the PE >1.3us and trigger a HAM re-throttle).
            ti = 0
            for img in range(B_LOC):
                for rg in range(N_RG):
                    last_pair = (img, rg) == (B_LOC - 1, N_RG - 1)
                    ps0 = psum.tile([128, 512], F32, tag="cps", bufs=5, name=f"cps{ti}")
                    if last_pair:
                        # final cb1 tile as two N=256 half-groups in their
                        # own PSUM tiles: half 0's drain+DMA overlap half
                        # 1's matmuls (N=256 streams ~half the cycles)
                        for k in range(9):
                            conv_mm(ps0, img, 0, rg, k)
                        drain(ps0, img, 0, rg, ti)
                        for half in range(2):
                            ph = psum.tile(
                                [128, 256], F32, tag=f"cpsh{half}",
                                bufs=1, name=f"cpsh{half}",
                            )
                            for k in range(9):
                                conv_mm(ph, img, 1, rg, k, half=half)
                            drain(ph, img, 1, rg, ti + 1, half=half)
                        ti += 2
                        continue
                    ps1 = psum.tile(
                        [128, 512], F32, tag="cps", bufs=5, name=f"cps{ti + 1}"
                    )
                    for k in range(9):
                        mm = conv_mm(ps0, img, 0, rg, k)
                        conv_mm(ps1, img, 1, rg, k)
                        # stage the bulk DMAs off early conv matmuls so they
                        # don't steal ring bandwidth at startup
                        if ti == 0 and k == 4:
                            xdma(nc.sync, 0, 2, after=mm)
                            xdma(nc.scalar, 0, 3, after=mm)
                        elif ti == 2 and k == 0:
                            xdma(nc.sync, 0, 4, after=mm)
                            xdma(nc.scalar, 0, 5, after=mm)
                        elif ti == 4 and k == 0:
                            for c in range(N_CHUNK):
                                xdma(nc.gpsimd, 1, c, after=mm)
                    drain(ps0, img, 0, rg, ti)
                    drain(ps1, img, 1, rg, ti + 1)
                    ti += 2

    nc.compile()
    return nc


_NC_CACHE = None


def _get_nc():
    global _NC_CACHE
    if _NC_CACHE is None:
        _NC_CACHE = _build_nc()
    return _NC_CACHE


def _host_prep(x, W, b, lora_A, lora_B):
    """Host input staging: pad + transpose + bf16 cast + LoRA weight fold."""
    bf16 = ml_dtypes.bfloat16
    x = np.asarray(x, dtype=np.float32)
    xp_all = np.zeros((B, C_IN, HP, WP), dtype=bf16)
    xp_all[:, :, 1 : H + 1, 1 : W_DIM + 1] = x.astype(bf16)
    xp_all = xp_all.reshape(B, C_IN, HP * WP)

    # W_eff = W + 2*(B@A), then [co, ci*9+k] -> [ci, k, co]
    weff = np.asarray(W, dtype=np.float32).reshape(C_OUT, C_IN * 9) + SCALING * (
        np.asarray(lora_B, dtype=np.float32) @ np.asarray(lora_A, dtype=np.float32)
    )
    we = np.ascontiguousarray(
        weff.reshape(C_OUT, C_IN, 9).transpose(1, 2, 0)
    ).reshape(C_IN, 9 * C_OUT).astype(bf16)
    # [256] -> [128, 2]: bv[p, cb] = b[cb*128 + p]
    bv = np.ascontiguousarray(np.asarray(b, dtype=np.float32).reshape(2, 128).T)
    return xp_all, we, bv


def run(x, W, b, lora_A, lora_B, trace=False):
    """Run the kernel on 8 cores; returns (full_output, BassKernelResults)."""
    xp_all, we, bv = _host_prep(x, W, b, lora_A, lora_B)
    nc = _get_nc()
    in_maps = []
    for c in range(N_CORES):
        in_maps.append(
            {
                "xp": np.ascontiguousarray(xp_all[c * B_LOC : (c + 1) * B_LOC]),
                "we": we,
                "bv": bv,
            }
        )
    res = run_bass_kernel_spmd(
        nc, in_maps, core_ids=list(range(N_CORES)), trace=trace
    )
    out = np.concatenate(
        [np.asarray(r["out"]).astype(np.float32) for r in res.results], axis=0
    )
    return out.reshape(B, C_OUT, H, W_DIM), res


def kernel(x, W, b, lora_A, lora_B):
    out, _ = run(x, W, b, lora_A, lora_B, trace=False)
    return out
